# revision 9
# baseline (speedup 1.0000x reference)
"""CP-decomposed 4D linear layer on 8 Trainium2 NeuronCores.

out[b, cls] = sum_r lam[r] * U4[cls,r] * sum_c U3[c,r] * sum_w U2[w,r] * sum_h U1[h,r] * x[b,c,w,h]

Strategy (data-parallel over batch, 16 b per core):
  - host precomputes G[r, w*32+h] = U2[w,r]*U1[h,r]  (64 x 1024, f32)
    and A[r, cls] = lam[r]*U4[cls,r]                  (64 x 1000, f32),
    casts x to bf16 laid out [b][p][k][f] with c = k*128 + p so each
    per-batch SBUF load is ONE fully-linear 1MB DMA (8KB per partition
    line), and pre-reorders U3 to [p][k][r] bf16 for a linear load.
  - per b: PE contracts c (K=512 as 4 accumulated chunks of 128):
        t[r, f] = sum_c U3[c,r] * x[b,c,f]   -> PSUM [64, 1024]  (bf16 matmul)
  - one fused DVE pass multiplies by G and reduces over f:
        z[r, b] = sum_f t[r,f]*G[r,f]        (tensor_tensor_reduce)
  - final PE matmul: out[b, cls] = sum_r zbuf[r,b] * A[r,cls]    (fp32)
  - x streams on the sync-engine DMA queue; constants + output use the
    scalar-engine queue so x descriptors start flowing immediately.
"""

import numpy as np
import ml_dtypes

import concourse.bass as bass
import concourse.bacc as bacc
import concourse.mybir as mybir
import concourse.tile as tile
from concourse.bass_utils import run_bass_kernel_spmd

B, C, W, H, CLS, R = 128, 512, 32, 32, 1000, 64
WH = W * H          # 1024
N_CORES = 8
B_LOC = B // N_CORES  # 16
KC = C // 128         # 4 contraction chunks
BF16 = mybir.dt.bfloat16
F32 = mybir.dt.float32

_NC_CACHE = {}


def _build(reps=1, xbufs=6, use_ttr="stt", single_dma=True, use_scalar_q=True):
    nc = bacc.Bacc()
    x = nc.declare_dram_parameter("x", [B_LOC, 128, KC, WH], BF16, isOutput=False)
    u3 = nc.declare_dram_parameter("u3", [128, KC, R], BF16, isOutput=False)
    g = nc.declare_dram_parameter("g", [R, WH], F32, isOutput=False)
    a = nc.declare_dram_parameter("a", [R, CLS], F32, isOutput=False)
    out = nc.declare_dram_parameter("out", [B_LOC, CLS], F32, isOutput=True)

    cq = nc.gpsimd if use_scalar_q else nc.sync

    with tile.TileContext(nc) as tc:
        with (
            tc.tile_pool(name="const", bufs=1) as cpool,
            tc.tile_pool(name="xp", bufs=xbufs) as xpool,
            tc.tile_pool(name="tmp", bufs=2) as tpool,
            tc.tile_pool(name="ps", bufs=3, space="PSUM") as pspool,
            tc.tile_pool(name="psd", bufs=1, space="PSUM") as psdpool,
        ):
            u3s = cpool.tile([128, KC, R], BF16)
            cq.dma_start(u3s[:], u3[:])
            gs = cpool.tile([R, WH], F32)
            cq.dma_start(gs[:], g[:])
            asb = cpool.tile([R, CLS], F32)
            cq.dma_start(asb[:], a[:])
            zbuf = cpool.tile([R, B_LOC], F32)

            for rep in range(reps):
                for b in range(B_LOC):
                    xb = xpool.tile([128, KC, WH], BF16, tag="xb")
                    if single_dma:
                        # two half-loads (k 0-1, k 2-3) on separate HWDGE
                        # queues: first matmuls start while the second half
                        # streams, and trigger issue rate doubles
                        nc.sync.dma_start(xb[:, 0:2, :], x[b, :, 0:2, :])
                        nc.scalar.dma_start(xb[:, 2:4, :], x[b, :, 2:4, :])
                    else:
                        for k in range(KC):
                            nc.sync.dma_start(xb[:, k, :], x[b, :, k, :])

                    tps = pspool.tile([64, WH], F32, tag="tps")
                    for k in range(KC):
                        for n in range(2):
                            sl = bass.ts(n, 512)
                            nc.tensor.matmul(
                                tps[:, sl],
                                u3s[:, k, :],
                                xb[:, k, sl],
                                start=(k == 0),
                                stop=(k == KC - 1),
                            )

                    if use_ttr == "stt":
                        # fused multiply+reduce via SCALAR_TENSOR_TENSOR:
                        # out = (tps * 1.0) * gs ; accum_out = sum(out)
                        tmp = tpool.tile([R, WH], F32, tag="ttr")
                        nc.vector.scalar_tensor_tensor(
                            tmp[:],
                            tps[:],
                            1.0,
                            gs[:],
                            mybir.AluOpType.mult,
                            mybir.AluOpType.mult,
                            accum_out=zbuf[:, b : b + 1],
                        )
                    elif use_ttr:
                        # qr.py-style: out is a stride-0 dummy (only accum_out
                        # is kept) — saves the full-size SBUF write.
                        dummy = tpool.tile([R, 1], F32, tag="ttr")
                        nc.vector.tensor_tensor_reduce(
                            dummy.broadcast_to((R, WH)),
                            gs[:],
                            tps[:],
                            1.0,
                            0.0,
                            mybir.AluOpType.mult,
                            mybir.AluOpType.add,
                            zbuf[:, b : b + 1],
                        )
                    else:
                        tmp = tpool.tile([R, WH], F32, tag="ttr")
                        nc.vector.tensor_tensor(
                            tmp[:], tps[:], gs[:], mybir.AluOpType.mult
                        )
                        nc.vector.tensor_reduce(
                            zbuf[:, b : b + 1],
                            tmp[:],
                            mybir.AxisListType.X,
                            mybir.AluOpType.add,
                        )

            # step D: out[b, cls] = sum_r zbuf[r, b] * A[r, cls]
            od0 = psdpool.tile([B_LOC, 512], F32, tag="od0")
            od1 = psdpool.tile([B_LOC, 512], F32, tag="od1")
            nc.tensor.matmul(od0[:], zbuf[:], asb[:, 0:512], start=True, stop=True)
            nc.tensor.matmul(
                od1[:, 0 : CLS - 512], zbuf[:], asb[:, 512:CLS], start=True, stop=True
            )
            osb = cpool.tile([B_LOC, CLS], F32)
            nc.vector.tensor_copy(osb[:, 0:512], od0[:])
            nc.vector.tensor_copy(osb[:, 512:CLS], od1[:, 0 : CLS - 512])
            cq.dma_start(out[:], osb[:])

    nc.compile()
    return nc


def _get_nc():
    if "v3" not in _NC_CACHE:
        _NC_CACHE["v3"] = _build()
    return _NC_CACHE["v3"]


def _prep_inputs(x, U1, U2, U3, U4, lam):
    x = np.asarray(x, dtype=np.float32).reshape(B, KC, 128, WH)
    U1 = np.asarray(U1, dtype=np.float32)
    U2 = np.asarray(U2, dtype=np.float32)
    U3 = np.asarray(U3, dtype=np.float32)
    U4 = np.asarray(U4, dtype=np.float32)
    lam = np.asarray(lam, dtype=np.float32)

    # x [B, k, p, f] -> [B, p, k, f] bf16 (c = k*128 + p): one linear 1MB
    # DMA per (core, b) with 8KB partition lines.
    xh = np.empty((B, 128, KC, WH), dtype=ml_dtypes.bfloat16)
    xh[...] = x.transpose(0, 2, 1, 3)
    # U3 [(k p), r] -> [p, k, r] bf16
    u3h = np.ascontiguousarray(
        U3.reshape(KC, 128, R).transpose(1, 0, 2)
    ).astype(ml_dtypes.bfloat16)
    # G[r, w*32+h] = U2[w,r] * U1[h,r]
    G = np.ascontiguousarray(
        (U2.T[:, :, None] * U1.T[:, None, :]).reshape(R, WH).astype(np.float32)
    )
    # A[r, cls] = lam[r] * U4[cls, r]
    A = np.ascontiguousarray((U4 * lam[None, :]).T.astype(np.float32))

    in_maps = [
        {
            "x": np.ascontiguousarray(xh[i * B_LOC : (i + 1) * B_LOC]),
            "u3": u3h,
            "g": G,
            "a": A,
        }
        for i in range(N_CORES)
    ]
    return in_maps


def kernel(x, U1, U2, U3, U4, lam):
    in_maps = _prep_inputs(x, U1, U2, U3, U4, lam)
    nc = _get_nc()
    res = run_bass_kernel_spmd(nc, in_maps, list(range(N_CORES)))
    return np.concatenate([res.results[i]["out"] for i in range(N_CORES)], axis=0)
